# revision 1
# baseline (speedup 1.0000x reference)
"""Per-pixel adaptive 5x5 conv (KPN) for Trainium2, 8-core data parallel.

out[g,h,w] = sum_{i,j} core[g,5i+j,h,w] * frames_pad[g,h+i-2,w+j-2]
with g = flattened (B,N) = 16 image planes; 2 planes per NeuronCore,
fused into one free dim (FD=4096) so every elementwise op covers both.

Engine split (GpSimd stays idle: it shares a physical SBUF port with
VectorE and concurrent streaming slows DVE tensor_tensor ~4.5x):
  DVE    - 25 products w_t*f_t (fp16 2x mode, 2.3us each); also
           dequantizes 6 weight tiles (tensor_copy int8->fp16): k=2 of
           each group plus tap 0, which rides DVE's dead ramp window
           while it waits for its first frame tile
  ACT    - dequantizes the other 19 weight tiles (activation-copy
           int8->fp16); evacuates PSUM fp32 -> SBUF fp16 at the end
  PE     - accumulates the 25 product streams into PSUM (fp32) via
           matmuls against a stationary (2^-5 * I); 8 banks = [128,4096]
  sync   - all DMAs, emitted in consume order; the identity load (128
           tiny descriptors) goes after group 0's batch so it doesn't
           clog the queue-ramp head

The DMA system is SBUF-WRITE-side limited (~27 GB/s per queue x 16;
measured: a casting DMA costs the same as its fp16 write size), so
weights are stored int8 (w8 = clip(round(w * 32), -127, 127)) and
DMA'd as int8 (queue writes 26.2 -> 13.1 MB/core) then cast to fp16
on chip by the otherwise-idle ACT engine (k=2 of each group is the
only steady-state arrival-safe DVE slot - the DMA stream runs only
~1 group ahead of compute, and later-k dequants stall DVE).
The 2^-5 dequant scale is folded into the PE's stationary identity,
so dequant is a pure cast.  Measured end-to-end rel err 9.4e-3
(gate 2e-2); HW exec 99.2-99.8us in the device's fast state
(103-105us under shared-device DMA jitter; op durations identical).

Host layouts:
  fin [5, 2, 128, 4144] fp16: fin[i, par, p, (img,blk,c)] =
     Fpad[img, blk*128+p+i, (1-par)+c], c in [0,518).  Parity copies
     keep every tap's 512-col slice 4-byte aligned for DVE 2x mode.
  win [25, 128, 4096] int8: win[t, p, (img,blk,c)] =
     clip(round(32*core[img, t, blk*128+p, c]), -127, 127)
  oout [128, 4096] fp16 (host casts to f32).
"""

import os
import sys

import numpy as np

for _p in ("/opt/trn_rl_repo",):
    if _p not in sys.path and os.path.isdir(_p):
        sys.path.insert(0, _p)

K = 5
NCORES = 8
IMGS_PER_CORE = 2
H = W = 512
NBLK = 4  # 128-row blocks per image
FCOLS = 518
F_FREE = IMGS_PER_CORE * NBLK * FCOLS  # 4144 per parity tile
W_FREE = IMGS_PER_CORE * NBLK * W  # 4096
O_FREE = IMGS_PER_CORE * NBLK * W  # 4096
NBANK = 8
BANK = O_FREE // NBANK  # 512 fp32 per PSUM bank
WSCALE = 2.0 ** -5  # int8 weight dequant scale, folded into PE identity

# All 25 product streams go straight to PE (it has headroom at ~2.7us
# per stream); weight dequant splits DVE 5 / ACT 20 so both engines
# land at ~72us: DVE 25 muls (57us) + 5 casts + PSUM evac, ACT 20
# casts (~3.6us each).
PAIR_K = ()
# t=0 rides DVE's dead ramp window (w8(0) is the earliest weight tile,
# arriving while DVE waits for its first frame tile anyway), shaving a
# tile off ACT's dense critical chain without mid-stream stall risk.
DVE_DEQ = frozenset({0, 2, 7, 12, 17, 22})
SWDGE_W = frozenset()

_compiled = {}
last_results = None  # BassKernelResults of the most recent run (for test.py)


def _build_nc():
    import concourse.bacc as bacc
    import concourse.mybir as mybir
    from concourse.tile import TileContext

    f16 = mybir.dt.float16
    f32 = mybir.dt.float32
    i8 = mybir.dt.int8

    nc = bacc.Bacc(None, target_bir_lowering=False, debug=False)
    ident = nc.dram_tensor("ident", [128, 128], f16, kind="ExternalInput")
    fin = nc.dram_tensor("fin", [K, 2, 128, F_FREE], f16,
                         kind="ExternalInput")
    win = nc.dram_tensor("win", [K * K, 128, W_FREE], i8,
                         kind="ExternalInput")
    oout = nc.dram_tensor("oout", [128, O_FREE], f16,
                          kind="ExternalOutput")

    n_streams = K * K

    with TileContext(nc) as tc:
        with (
            tc.tile_pool(name="ipool", bufs=1) as ipool,
            tc.tile_pool(name="fpool", bufs=2) as fpool,
            tc.tile_pool(name="w8pool", bufs=2) as w8pool,
            tc.tile_pool(name="wpool", bufs=2) as wpool,
            tc.tile_pool(name="spool", bufs=4) as spool,
            tc.tile_pool(name="opool", bufs=1) as opool,
            tc.tile_pool(name="ppool", bufs=1, space="PSUM") as ppool,
        ):
            id_t = ipool.tile([128, 128], f16, tag="ident")

            banks = [ppool.tile([128, BANK], f32, tag=f"b{b}",
                                name=f"bank{b}")
                     for b in range(NBANK)]
            osb = opool.tile([128, O_FREE], f16, tag="osb")

            f_tiles = {}
            w8_tiles = {}
            w_tiles = {}
            stream_idx = [0]

            def emit_w(tg, k):
                t = tg * K + k
                if t in SWDGE_W:
                    # casting DMA writes dequantized fp16 directly
                    w_t = wpool.tile([128, W_FREE], f16, tag=f"w{k}",
                                     name=f"wsw{t}")
                    nc.gpsimd.dma_start(out=w_t[:], in_=win[t])
                    w_tiles[t] = w_t
                    return
                w8_t = w8pool.tile([128, W_FREE], i8, tag=f"w8{k}",
                                   name=f"w8_{t}")
                nc.sync.dma_start(out=w8_t[:], in_=win[t])
                w8_tiles[t] = w8_t
                # ACT dequants (int8 codes -> fp16) are emitted here so
                # ACT runs a group ahead of the DVE muls; DVE's own
                # dequants are emitted inline in emit_compute.
                if t not in DVE_DEQ:
                    w_t = wpool.tile([128, W_FREE], f16, tag=f"w{k}",
                                     name=f"wdq{t}")
                    nc.scalar.copy(out=w_t[:], in_=w8_t[:])
                    w_tiles[t] = w_t

            def emit_dmas(tg):
                # first group: tap-0 path (f par0, w0) ahead of f par1
                f_t = fpool.tile([128, F_FREE], f16, tag="f0",
                                 name=f"fr{tg}p0")
                nc.sync.dma_start(out=f_t[:], in_=fin[tg, 0])
                f_tiles[(tg, 0)] = f_t
                emit_w(tg, 0)
                f_t = fpool.tile([128, F_FREE], f16, tag="f1",
                                 name=f"fr{tg}p1")
                nc.sync.dma_start(out=f_t[:], in_=fin[tg, 1])
                f_tiles[(tg, 1)] = f_t
                for k in range(1, K):
                    emit_w(tg, k)

            def pe_accumulate(tile):
                s = stream_idx[0]
                stream_idx[0] += 1
                for b in range(NBANK):
                    nc.tensor.matmul(
                        out=banks[b][:],
                        lhsT=id_t[:],
                        rhs=tile[:][:, b * BANK:(b + 1) * BANK],
                        start=(s == 0),
                        stop=(s == n_streams - 1),
                    )

            def emit_compute(tg):
                pair_tile = None
                for k in range(K):
                    t = tg * K + k
                    if t in DVE_DEQ:
                        w_t = wpool.tile([128, W_FREE], f16, tag=f"w{k}")
                        nc.vector.tensor_copy(out=w_t[:],
                                              in_=w8_tiles[t][:])
                        w_tiles[t] = w_t
                    j = k
                    par = j & 1
                    joff = j + par
                    fv = f_tiles[(tg, par)][:].rearrange(
                        "p (img blk c) -> p img blk c",
                        img=IMGS_PER_CORE, blk=NBLK, c=FCOLS)
                    f_ap = fv[:, :, :, joff:joff + W]
                    w_ap = w_tiles[t][:].rearrange(
                        "p (img blk c) -> p img blk c",
                        img=IMGS_PER_CORE, blk=NBLK, c=W)
                    tmp = spool.tile([128, O_FREE], f16, tag="s")
                    tv = tmp[:].rearrange("p (img blk c) -> p img blk c",
                                          img=IMGS_PER_CORE, blk=NBLK, c=W)
                    nc.vector.tensor_mul(out=tv, in0=w_ap, in1=f_ap)
                    if PAIR_K and k == PAIR_K[0]:
                        pair_tile = tmp
                    elif PAIR_K and k == PAIR_K[1]:
                        nc.vector.tensor_add(out=pair_tile[:],
                                             in0=pair_tile[:], in1=tmp[:])
                        pe_accumulate(pair_tile)
                    else:
                        pe_accumulate(tmp)

            emit_dmas(0)
            # identity loads behind group 0's tiles: its 128 tiny
            # descriptors would otherwise clog the queue-ramp head, and
            # the first matmul doesn't need it until ~25us in.
            nc.sync.dma_start(out=id_t[:], in_=ident[:])
            for tg in range(1, K):
                emit_dmas(tg)
                emit_compute(tg - 1)
            emit_compute(K - 1)

            # PSUM fp32 -> SBUF fp16 per bank on the (otherwise idle)
            # scalar engine, then store halves as they complete.
            for b in range(NBANK):
                nc.scalar.copy(out=osb[:][:, b * BANK:(b + 1) * BANK],
                               in_=banks[b][:])
                if b == NBANK // 2 - 1:
                    nc.sync.dma_start(out=oout[:, :O_FREE // 2],
                                      in_=osb[:][:, :O_FREE // 2])
            nc.sync.dma_start(out=oout[:, O_FREE // 2:],
                              in_=osb[:][:, O_FREE // 2:])
    nc.finalize()
    return nc


def _host_prep(frames, core):
    """Build per-core in_maps. frames [4,4,1,512,512] f32, core [4,4,25,1,512,512]."""
    G = NCORES * IMGS_PER_CORE  # 16
    F = np.ascontiguousarray(frames.reshape(G, H, W))
    Wc = core.reshape(G, K * K, H, W)

    # frames: pad rows 2/2, cols 3/4; Fp[g, r, c] = F[g, r-2, c-3]
    Fp = np.pad(F, ((0, 0), (2, 2), (3, 4))).astype(np.float16)
    # A[g, i, par, blk, p, c] = Fp[g, blk*128+p+i, (1-par)+c]
    A = np.empty((G, K, 2, NBLK, 128, FCOLS), np.float16)
    for i in range(K):
        for par in range(2):
            sl = Fp[:, i:i + H, (1 - par):(1 - par) + FCOLS]  # [G,512,518]
            A[:, i, par] = sl.reshape(G, NBLK, 128, FCOLS)
    # fin[core][i, par, p, (img, blk, c)]
    fprep = np.ascontiguousarray(
        A.reshape(NCORES, IMGS_PER_CORE, K, 2, NBLK, 128, FCOLS)
        .transpose(0, 2, 3, 5, 1, 4, 6))

    # win[core][t, p, (img, blk, c)] as int8 codes of w/2^-5
    w8 = np.clip(np.round(Wc.astype(np.float64) / WSCALE), -127, 127)
    w8 = w8.astype(np.int8).reshape(G, K * K, NBLK, 128, W)
    wprep = np.ascontiguousarray(
        w8.reshape(NCORES, IMGS_PER_CORE, K * K, NBLK, 128, W)
        .transpose(0, 2, 4, 1, 3, 5))

    ident = (np.eye(128) * WSCALE).astype(np.float16)
    in_maps = []
    for c in range(NCORES):
        in_maps.append({
            "ident": ident,
            "fin": fprep[c].reshape(K, 2, 128, F_FREE),
            "win": wprep[c].reshape(K * K, 128, W_FREE),
        })
    return in_maps


def kernel(frames, core, bias):
    global last_results
    from concourse.bass_utils import run_bass_kernel_spmd

    frames = np.asarray(frames, dtype=np.float32)
    core = np.asarray(core, dtype=np.float32)

    if "nc" not in _compiled:
        _compiled["nc"] = _build_nc()
    nc = _compiled["nc"]

    in_maps = _host_prep(frames, core)
    trace = os.environ.get("KC_TRACE") == "1"
    tmpdir = os.environ.get("KC_TRACE_DIR") or None
    if tmpdir:
        os.makedirs(tmpdir, exist_ok=True)
    res = run_bass_kernel_spmd(nc, in_maps, list(range(NCORES)), trace=trace,
                               tmpdir=tmpdir)
    last_results = res

    G = NCORES * IMGS_PER_CORE
    out = np.empty((G, H, W), np.float32)
    for c in range(NCORES):
        o = res.results[c]["oout"]  # [128, 4096] f16
        ov = o.reshape(128, IMGS_PER_CORE, NBLK, W).astype(np.float32)
        for img in range(IMGS_PER_CORE):
            out[c * IMGS_PER_CORE + img] = (
                ov[:, img].transpose(1, 0, 2).reshape(H, W))
    return out.reshape(4, 4, H, W)



# revision 2
# speedup vs baseline: 1.0469x; 1.0469x over previous
"""Per-pixel adaptive 5x5 conv (KPN) for Trainium2, 8-core data parallel.

out[g,h,w] = sum_{i,j} core[g,5i+j,h,w] * frames_pad[g,h+i-2,w+j-2]
with g = flattened (B,N) = 16 image planes; 2 planes per NeuronCore,
fused into one free dim so every elementwise op covers both.

v2 layout (vs v1's parity-copy scheme): each 128-row block stores 516
frame cols (out cols plus the +-2 halo), so ONE frame tile per row
shift i serves all 5 column taps j of its group: the column shift is
folded into the host weight layout (w'[c'] = w[c'-j]) and the PE reads
each product tile at free-dim offset j when accumulating bank b over
cols [b*516+j, b*516+j+512).  Frame DMA drops from 10.6 to 5.3 MB/core.

Engine split:
  DVE   - 25 products w_t*f_t (fp16 2x mode, ~2.2us each) plus 2
          weight dequants (t=0 in ramp shadow, t=12)
  ACT   - 15 weight dequants (int8 codes -> fp16 copy, 3.6us each)
  SWDGE - 8 weight tiles dequantized by casting DMA (int8 DRAM -> fp16
          SBUF), weighted toward late groups where the DMA queues have
          drained the input flood
  PE    - accumulates the 25 product streams into PSUM fp32 via
          matmuls against a stationary (2^-5 * I); 8 banks = [128,512]
  tail  - PSUM evac split scalar/vector engines, output stored in two
          0.5 MB chunks

Group 0 is split into per-image half tiles ([128,2064]) so the first
dequant+mul start as soon as ~0.25 MB lands instead of waiting for the
full 1.6 MB head (the DMA queues round-robin all in-flight transfers,
so first-tile latency is proportional to bytes in flight).

Weights are int8 (w8 = clip(round(w * 32), -127, 127)); the 2^-5
dequant scale is folded into the PE's stationary identity, so every
dequant path (ACT copy, DVE copy, casting DMA) is a pure cast.

Host layouts:
  fin [5, 128, 4128] fp16: fin[i][p, (img,blk,c)] =
     Fpad[img, blk*128+p+i, c], Fpad = pad(F, rows 2/2, cols 2/2),
     c in [0,516).
  win [25, 128, 4128] int8: win[t][p, (img,blk,c')] =
     clip(round(32*core[img, t, blk*128+p, c'-j]), -127, 127) for
     c'-j in [0,512) else 0, where j = t%5.
  oout [128, 4096] fp16 (host casts to f32).
"""

import os
import sys

import numpy as np

for _p in ("/opt/trn_rl_repo",):
    if _p not in sys.path and os.path.isdir(_p):
        sys.path.insert(0, _p)

K = 5
NCORES = 8
IMGS_PER_CORE = 2
H = W = 512
NBLK = 4          # 128-row blocks per image
C_BLK = 516       # 512 out cols + 4 halo cols (-2..513)
FREE = IMGS_PER_CORE * NBLK * C_BLK   # 4128
HFREE = FREE // 2                     # 2064 = one image
O_FREE = IMGS_PER_CORE * NBLK * W     # 4096
NBANK = 8
BANK = 512
WSCALE = 2.0 ** -5  # int8 weight dequant scale, folded into PE identity

# Dequant engine assignment per tap t = 5*i + j.
DVE_DEQ = frozenset({0, 12})
SWDGE_W = frozenset({4, 8, 9, 14, 18, 19, 23, 24})
# remaining 15 taps dequant on ACT

_compiled = {}
last_results = None  # BassKernelResults of the most recent run (for test.py)


def _build_nc():
    import concourse.bacc as bacc
    import concourse.mybir as mybir
    from concourse.tile import TileContext

    f16 = mybir.dt.float16
    f32 = mybir.dt.float32
    i8 = mybir.dt.int8

    nc = bacc.Bacc(None, target_bir_lowering=False, debug=False)
    ident = nc.dram_tensor("ident", [128, 128], f16, kind="ExternalInput")
    fin = nc.dram_tensor("fin", [K, 128, FREE], f16, kind="ExternalInput")
    win = nc.dram_tensor("win", [K * K, 128, FREE], i8, kind="ExternalInput")
    oout = nc.dram_tensor("oout", [128, O_FREE], f16, kind="ExternalOutput")

    n_streams = K * K

    with TileContext(nc) as tc:
        with (
            tc.tile_pool(name="ipool", bufs=1) as ipool,
            tc.tile_pool(name="fpool", bufs=2) as fpool,
            tc.tile_pool(name="fhpool", bufs=1) as fhpool,
            tc.tile_pool(name="w8pool", bufs=2) as w8pool,
            tc.tile_pool(name="whpool", bufs=1) as whpool,
            tc.tile_pool(name="wpool", bufs=2) as wpool,
            tc.tile_pool(name="spool", bufs=3) as spool,
            tc.tile_pool(name="shpool", bufs=2) as shpool,
            tc.tile_pool(name="opool", bufs=1) as opool,
            tc.tile_pool(name="ppool", bufs=1, space="PSUM") as ppool,
        ):
            id_t = ipool.tile([128, 128], f16, tag="ident")

            banks = [ppool.tile([128, BANK], f32, tag=f"b{b}",
                                name=f"bank{b}")
                     for b in range(NBANK)]
            osb = opool.tile([128, O_FREE], f16, tag="osb")

            f_tiles = {}
            w8_tiles = {}
            w_tiles = {}
            bank_n = [0] * NBANK

            def pe_acc(tile, j, bank_list, off0):
                # rhs covers out cols of bank b at product offset j
                for lb, b in enumerate(bank_list):
                    s = bank_n[b]
                    bank_n[b] += 1
                    nc.tensor.matmul(
                        out=banks[b][:],
                        lhsT=id_t[:],
                        rhs=tile[:][:, off0 + lb * C_BLK + j:
                                    off0 + lb * C_BLK + j + BANK],
                        start=(s == 0),
                        stop=(s == n_streams - 1),
                    )

            def emit_w(tg, k):
                t = tg * K + k
                if t in SWDGE_W:
                    # casting DMA writes dequantized fp16 directly
                    w_t = wpool.tile([128, FREE], f16, tag=f"w{k}",
                                     name=f"wsw{t}")
                    nc.gpsimd.dma_start(out=w_t[:], in_=win[t])
                    w_tiles[t] = w_t
                    return
                w8_t = w8pool.tile([128, FREE], i8, tag=f"w8{k}",
                                   name=f"w8_{t}")
                nc.sync.dma_start(out=w8_t[:], in_=win[t])
                w8_tiles[t] = w8_t
                # ACT dequants are emitted here so ACT chases the DMA
                # arrivals a group ahead of the DVE muls; DVE's own
                # dequants are emitted inline in emit_compute.
                if t not in DVE_DEQ:
                    w_t = wpool.tile([128, FREE], f16, tag=f"w{k}",
                                     name=f"wdq{t}")
                    nc.scalar.copy(out=w_t[:], in_=w8_t[:])
                    w_tiles[t] = w_t

            def emit_dmas(tg):
                f_t = fpool.tile([128, FREE], f16, tag="f",
                                 name=f"fr{tg}")
                nc.sync.dma_start(out=f_t[:], in_=fin[tg])
                f_tiles[tg] = f_t
                for k in range(K):
                    emit_w(tg, k)

            def emit_compute(tg):
                for k in range(K):
                    t = tg * K + k
                    if t in DVE_DEQ:
                        w_t = wpool.tile([128, FREE], f16, tag=f"w{k}",
                                         name=f"wdq{t}")
                        nc.vector.tensor_copy(out=w_t[:],
                                              in_=w8_tiles[t][:])
                        w_tiles[t] = w_t
                    tmp = spool.tile([128, FREE], f16, tag="s")
                    nc.vector.tensor_mul(out=tmp[:], in0=w_tiles[t][:],
                                         in1=f_tiles[tg][:])
                    pe_acc(tmp, k, range(NBANK), 0)

            # ---- head: group 0 in per-image halves for a fast ramp ----
            wh8 = []
            fh = []
            for h in range(2):
                wh8.append(whpool.tile([128, HFREE], i8, tag=f"wh8{h}",
                                       name=f"wh8_{h}"))
                fh.append(fhpool.tile([128, HFREE], f16, tag=f"fh{h}",
                                      name=f"fh{h}"))
            nc.sync.dma_start(out=wh8[0][:], in_=win[0][:, :HFREE])
            nc.sync.dma_start(out=fh[0][:], in_=fin[0][:, :HFREE])
            nc.sync.dma_start(out=wh8[1][:], in_=win[0][:, HFREE:])
            nc.sync.dma_start(out=fh[1][:], in_=fin[0][:, HFREE:])
            for k in range(1, K):
                emit_w(0, k)
            # identity rides the (empty) ACT hwdge ring, keeping its 128
            # tiny descriptors out of the sync ring's ramp window
            nc.scalar.dma_start(out=id_t[:], in_=ident[:])

            emit_dmas(1)

            # group 0 compute: DVE dequants t0 halves, then 10 half-muls
            wh16 = []
            for h in range(2):
                w_t = whpool.tile([128, HFREE], f16, tag=f"wh{h}",
                                  name=f"wh16_{h}")
                nc.vector.tensor_copy(out=w_t[:], in_=wh8[h][:])
                wh16.append(w_t)
            for k in range(K):
                for h in range(2):
                    if k == 0:
                        w_ap = wh16[h][:]
                    else:
                        w_ap = w_tiles[k][:][:, h * HFREE:(h + 1) * HFREE]
                    tmp = shpool.tile([128, HFREE], f16, tag="sh")
                    nc.vector.tensor_mul(out=tmp[:], in0=w_ap,
                                         in1=fh[h][:])
                    pe_acc(tmp, k, range(4 * h, 4 * h + 4), 0)

            for tg in range(2, K):
                emit_dmas(tg)
                emit_compute(tg - 1)
            emit_compute(K - 1)

            # ---- tail: PSUM fp32 -> SBUF fp16 split across scalar and
            # vector engines, store output in two chunks ----
            for b in range(NBANK):
                dst = osb[:][:, b * BANK:(b + 1) * BANK]
                if b % 2 == 0:
                    nc.scalar.copy(out=dst, in_=banks[b][:])
                else:
                    nc.vector.tensor_copy(out=dst, in_=banks[b][:])
                if b == NBANK // 2 - 1:
                    nc.sync.dma_start(out=oout[:, :O_FREE // 2],
                                      in_=osb[:][:, :O_FREE // 2])
            nc.sync.dma_start(out=oout[:, O_FREE // 2:],
                              in_=osb[:][:, O_FREE // 2:])
    nc.finalize()
    return nc


def _host_prep(frames, core):
    """Build per-core in_maps. frames [4,4,1,512,512] f32, core [4,4,25,1,512,512]."""
    G = NCORES * IMGS_PER_CORE  # 16
    F = np.ascontiguousarray(frames.reshape(G, H, W))
    Wc = core.reshape(G, K * K, H, W)

    # frames: pad rows 2/2, cols 2/2 -> [G, 516, 516]
    Fp = np.pad(F, ((0, 0), (2, 2), (2, 2))).astype(np.float16)
    # A[g, i, blk, p, c] = Fp[g, blk*128+p+i, c]
    A = np.empty((G, K, NBLK, 128, C_BLK), np.float16)
    for i in range(K):
        A[:, i] = Fp[:, i:i + H, :].reshape(G, NBLK, 128, C_BLK)
    # fin[core][i, p, (img, blk, c)]
    fprep = np.ascontiguousarray(
        A.reshape(NCORES, IMGS_PER_CORE, K, NBLK, 128, C_BLK)
        .transpose(0, 2, 4, 1, 3, 5))

    # weights: int8 codes, column-shifted by j so products line up with
    # an aligned frame read; PE reads the product at offset j.
    w8 = np.clip(np.round(Wc * (1.0 / WSCALE)), -127, 127).astype(np.int8)
    Ws = np.zeros((G, K * K, H, C_BLK), np.int8)
    for j in range(K):
        Ws[:, j::K, :, j:j + W] = w8[:, j::K]
    wprep = np.ascontiguousarray(
        Ws.reshape(NCORES, IMGS_PER_CORE, K * K, NBLK, 128, C_BLK)
        .transpose(0, 2, 4, 1, 3, 5))

    ident = (np.eye(128) * WSCALE).astype(np.float16)
    in_maps = []
    for c in range(NCORES):
        in_maps.append({
            "ident": ident,
            "fin": fprep[c].reshape(K, 128, FREE),
            "win": wprep[c].reshape(K * K, 128, FREE),
        })
    return in_maps


def kernel(frames, core, bias):
    global last_results
    from concourse.bass_utils import run_bass_kernel_spmd

    frames = np.asarray(frames, dtype=np.float32)
    core = np.asarray(core, dtype=np.float32)

    if "nc" not in _compiled:
        _compiled["nc"] = _build_nc()
    nc = _compiled["nc"]

    in_maps = _host_prep(frames, core)
    trace = os.environ.get("KC_TRACE") == "1"
    tmpdir = os.environ.get("KC_TRACE_DIR") or None
    if tmpdir:
        os.makedirs(tmpdir, exist_ok=True)
    res = run_bass_kernel_spmd(nc, in_maps, list(range(NCORES)), trace=trace,
                               tmpdir=tmpdir)
    last_results = res

    G = NCORES * IMGS_PER_CORE
    out = np.empty((G, H, W), np.float32)
    for c in range(NCORES):
        o = res.results[c]["oout"]  # [128, 4096] f16
        ov = o.reshape(128, IMGS_PER_CORE, NBLK, W).astype(np.float32)
        for img in range(IMGS_PER_CORE):
            out[c * IMGS_PER_CORE + img] = (
                ov[:, img].transpose(1, 0, 2).reshape(H, W))
    return out.reshape(4, 4, H, W)


# revision 8
# speedup vs baseline: 1.1829x; 1.1299x over previous
"""Per-pixel adaptive 5x5 conv (KPN) for Trainium2, 8-core data parallel.

out[g,h,w] = sum_{i,j} core[g,5i+j,h,w] * frames_pad[g,h+i-2,w+j-2]
with g = flattened (B,N) = 16 image planes; 2 planes per NeuronCore,
fused into one free dim so every elementwise op covers both.

v2 layout (vs v1's parity-copy scheme): each 128-row block stores 516
frame cols (out cols plus the +-2 halo), so ONE frame tile per row
shift i serves all 5 column taps j of its group: the column shift is
folded into the host weight layout (w'[c'] = w[c'-j]) and the PE reads
each product tile at free-dim offset j when accumulating bank b over
cols [b*516+j, b*516+j+512).  Frame DMA drops from 10.6 to 5.3 MB/core.

Engine split:
  DVE   - 25 products w_t*f_t (fp16 2x mode, ~2.2us each) plus the
          t=0 weight dequant (rides the ramp shadow)
  ACT   - 15 weight dequants (int8 codes -> fp16 copy, 3.6us each)
  DMA   - 9 weight tiles stored as fp16 codes in DRAM and loaded
          directly (no dequant anywhere; costs +0.53 MB of DMA each,
          cheaper than an engine cast while the queues have slack;
          SWDGE casting DMA was measured to cost read+write on the
          queues, worse than both)
  PE    - accumulates the 25 product streams into PSUM fp32 via
          matmuls against a stationary (2^-5 * I); 8 banks = [128,512]
  tail  - PSUM evac split scalar/vector engines, output stored in two
          0.5 MB chunks

Group 0 is split into per-image half tiles ([128,2064]) so the first
dequant+mul start as soon as ~0.25 MB lands instead of waiting for the
full 1.6 MB head (the DMA queues round-robin all in-flight transfers,
so first-tile latency is proportional to bytes in flight).

Weights are int8 (w8 = clip(round(w * 32), -127, 127)); the 2^-5
dequant scale is folded into the PE's stationary identity, so every
dequant path (ACT copy, DVE copy, casting DMA) is a pure cast.

Host layouts:
  fin [5, 128, 4128] fp16: fin[i][p, (img,blk,c)] =
     Fpad[img, blk*128+p+i, c], Fpad = pad(F, rows 2/2, cols 2/2),
     c in [0,516).
  win [25, 128, 4128] int8: win[t][p, (img,blk,c')] =
     clip(round(32*core[img, t, blk*128+p, c'-j]), -127, 127) for
     c'-j in [0,512) else 0, where j = t%5.
  oout [128, 4096] fp16 (host casts to f32).
"""

import os
import sys

import numpy as np

for _p in ("/opt/trn_rl_repo",):
    if _p not in sys.path and os.path.isdir(_p):
        sys.path.insert(0, _p)

K = 5
NCORES = 8
IMGS_PER_CORE = 2
H = W = 512
NBLK = 4          # 128-row blocks per image
C_BLK = 516       # 512 out cols + 4 halo cols (-2..513)
FREE = IMGS_PER_CORE * NBLK * C_BLK   # 4128
HFREE = FREE // 2                     # 2064 = one image
O_FREE = IMGS_PER_CORE * NBLK * W     # 4096
NBANK = 8
BANK = 512
WSCALE = 2.0 ** -5  # int8 weight dequant scale, folded into PE identity

# Dequant engine assignment per tap t = 5*i + j.
DVE_DEQ = frozenset({0})
FP16_W = (4, 8, 9, 13, 14, 18, 19, 23, 24)  # stored fp16, no dequant
FP16_IDX = {t: n for n, t in enumerate(FP16_W)}
# remaining 15 taps dequant on ACT

_compiled = {}
last_results = None  # BassKernelResults of the most recent run (for test.py)


def _build_nc():
    import concourse.bacc as bacc
    import concourse.mybir as mybir
    from concourse.tile import TileContext

    f16 = mybir.dt.float16
    f32 = mybir.dt.float32
    i8 = mybir.dt.int8

    nc = bacc.Bacc(None, target_bir_lowering=False, debug=False)
    ident = nc.dram_tensor("ident", [128, 128], f16, kind="ExternalInput")
    fin = nc.dram_tensor("fin", [K, 128, FREE], f16, kind="ExternalInput")
    win = nc.dram_tensor("win", [K * K, 128, FREE], i8, kind="ExternalInput")
    win16 = nc.dram_tensor("win16", [len(FP16_W), 128, FREE], f16,
                           kind="ExternalInput")
    oout = nc.dram_tensor("oout", [128, O_FREE], f16, kind="ExternalOutput")

    n_streams = K * K

    with TileContext(nc) as tc:
        with (
            tc.tile_pool(name="ipool", bufs=1) as ipool,
            tc.tile_pool(name="fpool", bufs=3) as fpool,
            tc.tile_pool(name="fhpool", bufs=1) as fhpool,
            tc.tile_pool(name="w8pool", bufs=2) as w8pool,
            tc.tile_pool(name="whpool", bufs=1) as whpool,
            tc.tile_pool(name="wpool", bufs=2) as wpool,
            tc.tile_pool(name="spool", bufs=3) as spool,
            tc.tile_pool(name="shpool", bufs=2) as shpool,
            tc.tile_pool(name="opool", bufs=1) as opool,
            tc.tile_pool(name="ppool", bufs=1, space="PSUM") as ppool,
        ):
            id_t = ipool.tile([128, 128], f16, tag="ident")

            banks = [ppool.tile([128, BANK], f32, tag=f"b{b}",
                                name=f"bank{b}")
                     for b in range(NBANK)]
            osb = opool.tile([128, O_FREE], f16, tag="osb")

            f_tiles = {}
            w8_tiles = {}
            w_tiles = {}
            bank_n = [0] * NBANK

            def pe_acc(tile, j, bank_list, off0):
                # rhs covers out cols of bank b at product offset j
                for lb, b in enumerate(bank_list):
                    s = bank_n[b]
                    bank_n[b] += 1
                    nc.tensor.matmul(
                        out=banks[b][:],
                        lhsT=id_t[:],
                        rhs=tile[:][:, off0 + lb * C_BLK + j:
                                    off0 + lb * C_BLK + j + BANK],
                        start=(s == 0),
                        stop=(s == n_streams - 1),
                    )

            def emit_w(tg, k):
                t = tg * K + k
                if t in FP16_IDX:
                    # fp16 codes straight from DRAM, no dequant step
                    w_t = wpool.tile([128, FREE], f16, tag=f"w{k}",
                                     name=f"wf16_{t}")
                    nc.sync.dma_start(out=w_t[:], in_=win16[FP16_IDX[t]])
                    w_tiles[t] = w_t
                    return
                w8_t = w8pool.tile([128, FREE], i8, tag=f"w8{k}",
                                   name=f"w8_{t}")
                nc.sync.dma_start(out=w8_t[:], in_=win[t])
                w8_tiles[t] = w8_t
                # ACT dequants are emitted here so ACT chases the DMA
                # arrivals a group ahead of the DVE muls; DVE's own
                # dequants are emitted inline in emit_compute.
                if t not in DVE_DEQ:
                    w_t = wpool.tile([128, FREE], f16, tag=f"w{k}",
                                     name=f"wdq{t}")
                    nc.scalar.copy(out=w_t[:], in_=w8_t[:])
                    w_tiles[t] = w_t

            def emit_dmas(tg):
                f_t = fpool.tile([128, FREE], f16, tag="f",
                                 name=f"fr{tg}")
                nc.sync.dma_start(out=f_t[:], in_=fin[tg])
                f_tiles[tg] = f_t
                for k in range(K):
                    emit_w(tg, k)

            def emit_compute(tg):
                for k in range(K):
                    t = tg * K + k
                    if t in DVE_DEQ:
                        w_t = wpool.tile([128, FREE], f16, tag=f"w{k}",
                                         name=f"wdq{t}")
                        nc.vector.tensor_copy(out=w_t[:],
                                              in_=w8_tiles[t][:])
                        w_tiles[t] = w_t
                    tmp = spool.tile([128, FREE], f16, tag="s")
                    nc.vector.tensor_mul(out=tmp[:], in0=w_tiles[t][:],
                                         in1=f_tiles[tg][:])
                    pe_acc(tmp, k, range(NBANK), 0)

            # ---- head: group 0 in per-image halves for a fast ramp ----
            wh8 = []
            fh = []
            for h in range(2):
                wh8.append(whpool.tile([128, HFREE], i8, tag=f"wh8{h}",
                                       name=f"wh8_{h}"))
                fh.append(fhpool.tile([128, HFREE], f16, tag=f"fh{h}",
                                      name=f"fh{h}"))
            nc.sync.dma_start(out=wh8[0][:], in_=win[0][:, :HFREE])
            nc.sync.dma_start(out=fh[0][:], in_=fin[0][:, :HFREE])
            nc.sync.dma_start(out=wh8[1][:], in_=win[0][:, HFREE:])
            nc.sync.dma_start(out=fh[1][:], in_=fin[0][:, HFREE:])
            for k in range(1, K):
                emit_w(0, k)
            # identity rides the (empty) ACT hwdge ring, keeping its 128
            # tiny descriptors out of the sync ring's ramp window
            nc.scalar.dma_start(out=id_t[:], in_=ident[:])

            emit_dmas(1)

            # group 0 compute: DVE dequants t0 halves, then 10 half-muls
            wh16 = []
            for h in range(2):
                w_t = whpool.tile([128, HFREE], f16, tag=f"wh{h}",
                                  name=f"wh16_{h}")
                nc.vector.tensor_copy(out=w_t[:], in_=wh8[h][:])
                wh16.append(w_t)
            for k in range(K):
                for h in range(2):
                    if k == 0:
                        w_ap = wh16[h][:]
                    else:
                        w_ap = w_tiles[k][:][:, h * HFREE:(h + 1) * HFREE]
                    tmp = shpool.tile([128, HFREE], f16, tag="sh")
                    nc.vector.tensor_mul(out=tmp[:], in0=w_ap,
                                         in1=fh[h][:])
                    pe_acc(tmp, k, range(4 * h, 4 * h + 4), 0)

            for tg in range(2, K):
                emit_dmas(tg)
                emit_compute(tg - 1)
            emit_compute(K - 1)

            # ---- tail: PSUM fp32 -> SBUF fp16 split across scalar and
            # vector engines, store output in two chunks ----
            for b in range(NBANK):
                dst = osb[:][:, b * BANK:(b + 1) * BANK]
                if b % 2 == 0:
                    nc.scalar.copy(out=dst, in_=banks[b][:])
                else:
                    nc.vector.tensor_copy(out=dst, in_=banks[b][:])
                if b == NBANK // 2 - 1:
                    nc.sync.dma_start(out=oout[:, :O_FREE // 2],
                                      in_=osb[:][:, :O_FREE // 2])
            nc.sync.dma_start(out=oout[:, O_FREE // 2:],
                              in_=osb[:][:, O_FREE // 2:])
    nc.finalize()
    return nc


def _host_prep(frames, core):
    """Build per-core in_maps. frames [4,4,1,512,512] f32, core [4,4,25,1,512,512]."""
    G = NCORES * IMGS_PER_CORE  # 16
    F = np.ascontiguousarray(frames.reshape(G, H, W))
    Wc = core.reshape(G, K * K, H, W)

    # frames: pad rows 2/2, cols 2/2 -> [G, 516, 516]
    Fp = np.pad(F, ((0, 0), (2, 2), (2, 2))).astype(np.float16)
    # A[g, i, blk, p, c] = Fp[g, blk*128+p+i, c]
    A = np.empty((G, K, NBLK, 128, C_BLK), np.float16)
    for i in range(K):
        A[:, i] = Fp[:, i:i + H, :].reshape(G, NBLK, 128, C_BLK)
    # fin[core][i, p, (img, blk, c)]
    fprep = np.ascontiguousarray(
        A.reshape(NCORES, IMGS_PER_CORE, K, NBLK, 128, C_BLK)
        .transpose(0, 2, 4, 1, 3, 5))

    # weights: codes of w/2^-5, column-shifted by j so products line up
    # with an aligned frame read; PE reads the product at offset j.
    # int8-rounded codes for the engine-cast taps, full fp16 codes for
    # the direct-load taps.
    w8 = np.clip(np.round(Wc * (1.0 / WSCALE)), -127, 127).astype(np.int8)
    Ws = np.zeros((G, K * K, H, C_BLK), np.int8)
    for j in range(K):
        Ws[:, j::K, :, j:j + W] = w8[:, j::K]
    wprep = np.ascontiguousarray(
        Ws.reshape(NCORES, IMGS_PER_CORE, K * K, NBLK, 128, C_BLK)
        .transpose(0, 2, 4, 1, 3, 5))

    tf = list(FP16_W)
    Wf = np.zeros((G, len(tf), H, C_BLK), np.float16)
    for n, t in enumerate(tf):
        j = t % K
        Wf[:, n, :, j:j + W] = (Wc[:, t] * (1.0 / WSCALE)).astype(np.float16)
    wfprep = np.ascontiguousarray(
        Wf.reshape(NCORES, IMGS_PER_CORE, len(tf), NBLK, 128, C_BLK)
        .transpose(0, 2, 4, 1, 3, 5))

    ident = (np.eye(128) * WSCALE).astype(np.float16)
    in_maps = []
    for c in range(NCORES):
        in_maps.append({
            "ident": ident,
            "fin": fprep[c].reshape(K, 128, FREE),
            "win": wprep[c].reshape(K * K, 128, FREE),
            "win16": wfprep[c].reshape(len(tf), 128, FREE),
        })
    return in_maps


def kernel(frames, core, bias):
    global last_results
    from concourse.bass_utils import run_bass_kernel_spmd

    frames = np.asarray(frames, dtype=np.float32)
    core = np.asarray(core, dtype=np.float32)

    if "nc" not in _compiled:
        _compiled["nc"] = _build_nc()
    nc = _compiled["nc"]

    in_maps = _host_prep(frames, core)
    trace = os.environ.get("KC_TRACE") == "1"
    tmpdir = os.environ.get("KC_TRACE_DIR") or None
    if tmpdir:
        os.makedirs(tmpdir, exist_ok=True)
    res = run_bass_kernel_spmd(nc, in_maps, list(range(NCORES)), trace=trace,
                               tmpdir=tmpdir)
    last_results = res

    G = NCORES * IMGS_PER_CORE
    out = np.empty((G, H, W), np.float32)
    for c in range(NCORES):
        o = res.results[c]["oout"]  # [128, 4096] f16
        ov = o.reshape(128, IMGS_PER_CORE, NBLK, W).astype(np.float32)
        for img in range(IMGS_PER_CORE):
            out[c * IMGS_PER_CORE + img] = (
                ov[:, img].transpose(1, 0, 2).reshape(H, W))
    return out.reshape(4, 4, H, W)


# revision 11
# speedup vs baseline: 1.1837x; 1.0007x over previous
"""Per-pixel adaptive 5x5 conv (KPN) for Trainium2, 8-core data parallel.

out[g,h,w] = sum_{i,j} core[g,5i+j,h,w] * frames_pad[g,h+i-2,w+j-2]
with g = flattened (B,N) = 16 image planes; 2 planes per NeuronCore,
fused into one free dim so every elementwise op covers both.

v2 layout (vs v1's parity-copy scheme): each 128-row block stores 516
frame cols (out cols plus the +-2 halo), so ONE frame tile per row
shift i serves all 5 column taps j of its group: the column shift is
folded into the host weight layout (w'[c'] = w[c'-j]) and the PE reads
each product tile at free-dim offset j when accumulating bank b over
cols [b*516+j, b*516+j+512).  Frame DMA drops from 10.6 to 5.3 MB/core.

Engine split:
  DVE   - 25 products w_t*f_t (fp16 2x mode, ~2.2us each) plus the
          t=0 weight dequant (rides the ramp shadow)
  ACT   - 15 weight dequants (int8 codes -> fp16 copy, 3.6us each)
  DMA   - 9 weight tiles stored as fp16 codes in DRAM and loaded
          directly (no dequant anywhere; costs +0.53 MB of DMA each,
          cheaper than an engine cast while the queues have slack;
          SWDGE casting DMA was measured to cost read+write on the
          queues, worse than both)
  PE    - accumulates the 25 product streams into PSUM fp32 via
          matmuls against a stationary (2^-5 * I); 8 banks = [128,512]
  tail  - PSUM evac split scalar/vector engines, output stored in two
          0.5 MB chunks

Group 0 is split into per-image half tiles ([128,2064]) so the first
dequant+mul start as soon as ~0.25 MB lands instead of waiting for the
full 1.6 MB head (the DMA queues round-robin all in-flight transfers,
so first-tile latency is proportional to bytes in flight).

Weights are int8 (w8 = clip(round(w * 32), -127, 127)); the 2^-5
dequant scale is folded into the PE's stationary identity, so every
dequant path (ACT copy, DVE copy, casting DMA) is a pure cast.

Host layouts:
  fin [5, 128, 4128] fp16: fin[i][p, (img,blk,c)] =
     Fpad[img, blk*128+p+i, c], Fpad = pad(F, rows 2/2, cols 2/2),
     c in [0,516).
  win [25, 128, 4128] int8: win[t][p, (img,blk,c')] =
     clip(round(32*core[img, t, blk*128+p, c'-j]), -127, 127) for
     c'-j in [0,512) else 0, where j = t%5.
  oout [128, 4096] fp16 (host casts to f32).
"""

import os
import sys

import numpy as np

for _p in ("/opt/trn_rl_repo",):
    if _p not in sys.path and os.path.isdir(_p):
        sys.path.insert(0, _p)

K = 5
NCORES = 8
IMGS_PER_CORE = 2
H = W = 512
NBLK = 4          # 128-row blocks per image
C_BLK = 516       # 512 out cols + 4 halo cols (-2..513)
FREE = IMGS_PER_CORE * NBLK * C_BLK   # 4128
HFREE = FREE // 2                     # 2064 = one image
O_FREE = IMGS_PER_CORE * NBLK * W     # 4096
NBANK = 8
BANK = 512
WSCALE = 2.0 ** -5  # int8 weight dequant scale, folded into PE identity

# Dequant engine assignment per tap t = 5*i + j.
DVE_DEQ = frozenset({0})
FP16_W = (1, 4, 8, 9, 13, 14, 18, 19, 23, 24)  # stored fp16, no dequant
FP16_IDX = {t: n for n, t in enumerate(FP16_W)}
# remaining 15 taps dequant on ACT

_compiled = {}
last_results = None  # BassKernelResults of the most recent run (for test.py)


def _build_nc():
    import concourse.bacc as bacc
    import concourse.mybir as mybir
    from concourse.tile import TileContext

    f16 = mybir.dt.float16
    f32 = mybir.dt.float32
    i8 = mybir.dt.int8

    nc = bacc.Bacc(None, target_bir_lowering=False, debug=False)
    ident = nc.dram_tensor("ident", [128, 128], f16, kind="ExternalInput")
    fin = nc.dram_tensor("fin", [K, 128, FREE], f16, kind="ExternalInput")
    win = nc.dram_tensor("win", [K * K, 128, FREE], i8, kind="ExternalInput")
    win16 = nc.dram_tensor("win16", [len(FP16_W), 128, FREE], f16,
                           kind="ExternalInput")
    oout = nc.dram_tensor("oout", [128, O_FREE], f16, kind="ExternalOutput")

    n_streams = K * K

    with TileContext(nc) as tc:
        with (
            tc.tile_pool(name="ipool", bufs=1) as ipool,
            tc.tile_pool(name="fpool", bufs=3) as fpool,
            tc.tile_pool(name="fhpool", bufs=1) as fhpool,
            tc.tile_pool(name="w8pool", bufs=2) as w8pool,
            tc.tile_pool(name="whpool", bufs=1) as whpool,
            tc.tile_pool(name="wpool", bufs=2) as wpool,
            tc.tile_pool(name="spool", bufs=3) as spool,
            tc.tile_pool(name="shpool", bufs=2) as shpool,
            tc.tile_pool(name="opool", bufs=1) as opool,
            tc.tile_pool(name="ppool", bufs=1, space="PSUM") as ppool,
        ):
            id_t = ipool.tile([128, 128], f16, tag="ident")

            banks = [ppool.tile([128, BANK], f32, tag=f"b{b}",
                                name=f"bank{b}")
                     for b in range(NBANK)]
            osb = opool.tile([128, O_FREE], f16, tag="osb")

            f_tiles = {}
            w8_tiles = {}
            w_tiles = {}
            bank_n = [0] * NBANK

            def pe_acc(tile, j, bank_list, off0):
                # rhs covers out cols of bank b at product offset j
                for lb, b in enumerate(bank_list):
                    s = bank_n[b]
                    bank_n[b] += 1
                    nc.tensor.matmul(
                        out=banks[b][:],
                        lhsT=id_t[:],
                        rhs=tile[:][:, off0 + lb * C_BLK + j:
                                    off0 + lb * C_BLK + j + BANK],
                        start=(s == 0),
                        stop=(s == n_streams - 1),
                    )

            def emit_w(tg, k):
                t = tg * K + k
                if t in FP16_IDX:
                    # fp16 codes straight from DRAM, no dequant step
                    w_t = wpool.tile([128, FREE], f16, tag=f"w{k}",
                                     name=f"wf16_{t}")
                    nc.sync.dma_start(out=w_t[:], in_=win16[FP16_IDX[t]])
                    w_tiles[t] = w_t
                    return
                w8_t = w8pool.tile([128, FREE], i8, tag=f"w8{k}",
                                   name=f"w8_{t}")
                nc.sync.dma_start(out=w8_t[:], in_=win[t])
                w8_tiles[t] = w8_t
                # ACT dequants are emitted here so ACT chases the DMA
                # arrivals a group ahead of the DVE muls; DVE's own
                # dequants are emitted inline in emit_compute.
                if t not in DVE_DEQ:
                    w_t = wpool.tile([128, FREE], f16, tag=f"w{k}",
                                     name=f"wdq{t}")
                    nc.scalar.copy(out=w_t[:], in_=w8_t[:])
                    w_tiles[t] = w_t

            def emit_dmas(tg):
                f_t = fpool.tile([128, FREE], f16, tag="f",
                                 name=f"fr{tg}")
                nc.sync.dma_start(out=f_t[:], in_=fin[tg])
                f_tiles[tg] = f_t
                for k in range(K):
                    emit_w(tg, k)

            def emit_compute(tg):
                for k in range(K):
                    t = tg * K + k
                    if t in DVE_DEQ:
                        w_t = wpool.tile([128, FREE], f16, tag=f"w{k}",
                                         name=f"wdq{t}")
                        nc.vector.tensor_copy(out=w_t[:],
                                              in_=w8_tiles[t][:])
                        w_tiles[t] = w_t
                    if t == n_streams - 1:
                        # final tap in halves so the PSUM evac + store
                        # tail starts ~1us earlier
                        for h in range(2):
                            sl = slice(h * HFREE, (h + 1) * HFREE)
                            tmp = shpool.tile([128, HFREE], f16, tag="sh")
                            nc.vector.tensor_mul(
                                out=tmp[:], in0=w_tiles[t][:][:, sl],
                                in1=f_tiles[tg][:][:, sl])
                            pe_acc(tmp, k, range(4 * h, 4 * h + 4), 0)
                        continue
                    tmp = spool.tile([128, FREE], f16, tag="s")
                    nc.vector.tensor_mul(out=tmp[:], in0=w_tiles[t][:],
                                         in1=f_tiles[tg][:])
                    pe_acc(tmp, k, range(NBANK), 0)

            # ---- head: group 0 in per-image halves for a fast ramp ----
            wh8 = []
            fh = []
            for h in range(2):
                wh8.append(whpool.tile([128, HFREE], i8, tag=f"wh8{h}",
                                       name=f"wh8_{h}"))
                fh.append(fhpool.tile([128, HFREE], f16, tag=f"fh{h}",
                                      name=f"fh{h}"))
            nc.sync.dma_start(out=wh8[0][:], in_=win[0][:, :HFREE])
            nc.sync.dma_start(out=fh[0][:], in_=fin[0][:, :HFREE])
            nc.sync.dma_start(out=wh8[1][:], in_=win[0][:, HFREE:])
            nc.sync.dma_start(out=fh[1][:], in_=fin[0][:, HFREE:])
            for k in range(1, K):
                emit_w(0, k)
            # identity rides the (empty) ACT hwdge ring, keeping its 128
            # tiny descriptors out of the sync ring's ramp window
            nc.scalar.dma_start(out=id_t[:], in_=ident[:])

            emit_dmas(1)

            # group 0 compute: DVE dequants t0 halves, then 10 half-muls
            wh16 = []
            for h in range(2):
                w_t = whpool.tile([128, HFREE], f16, tag=f"wh{h}",
                                  name=f"wh16_{h}")
                nc.vector.tensor_copy(out=w_t[:], in_=wh8[h][:])
                wh16.append(w_t)
            for k in range(K):
                for h in range(2):
                    if k == 0:
                        w_ap = wh16[h][:]
                    else:
                        w_ap = w_tiles[k][:][:, h * HFREE:(h + 1) * HFREE]
                    tmp = shpool.tile([128, HFREE], f16, tag="sh")
                    nc.vector.tensor_mul(out=tmp[:], in0=w_ap,
                                         in1=fh[h][:])
                    pe_acc(tmp, k, range(4 * h, 4 * h + 4), 0)

            for tg in range(2, K):
                emit_dmas(tg)
                emit_compute(tg - 1)
            emit_compute(K - 1)

            # ---- tail: PSUM fp32 -> SBUF fp16 split across scalar and
            # vector engines, store output in two chunks ----
            for b in range(NBANK):
                dst = osb[:][:, b * BANK:(b + 1) * BANK]
                if b % 2 == 0:
                    nc.scalar.copy(out=dst, in_=banks[b][:])
                else:
                    nc.vector.tensor_copy(out=dst, in_=banks[b][:])
                if b % 2 == 1:
                    c0 = (b - 1) * BANK
                    nc.sync.dma_start(out=oout[:, c0:c0 + 2 * BANK],
                                      in_=osb[:][:, c0:c0 + 2 * BANK])
    nc.finalize()
    return nc


def _host_prep(frames, core):
    """Build per-core in_maps. frames [4,4,1,512,512] f32, core [4,4,25,1,512,512]."""
    G = NCORES * IMGS_PER_CORE  # 16
    F = np.ascontiguousarray(frames.reshape(G, H, W))
    Wc = core.reshape(G, K * K, H, W)

    # frames: pad rows 2/2, cols 2/2 -> [G, 516, 516]
    Fp = np.pad(F, ((0, 0), (2, 2), (2, 2))).astype(np.float16)
    # A[g, i, blk, p, c] = Fp[g, blk*128+p+i, c]
    A = np.empty((G, K, NBLK, 128, C_BLK), np.float16)
    for i in range(K):
        A[:, i] = Fp[:, i:i + H, :].reshape(G, NBLK, 128, C_BLK)
    # fin[core][i, p, (img, blk, c)]
    fprep = np.ascontiguousarray(
        A.reshape(NCORES, IMGS_PER_CORE, K, NBLK, 128, C_BLK)
        .transpose(0, 2, 4, 1, 3, 5))

    # weights: codes of w/2^-5, column-shifted by j so products line up
    # with an aligned frame read; PE reads the product at offset j.
    # int8-rounded codes for the engine-cast taps, full fp16 codes for
    # the direct-load taps.
    w8 = np.clip(np.round(Wc * (1.0 / WSCALE)), -127, 127).astype(np.int8)
    Ws = np.zeros((G, K * K, H, C_BLK), np.int8)
    for j in range(K):
        Ws[:, j::K, :, j:j + W] = w8[:, j::K]
    wprep = np.ascontiguousarray(
        Ws.reshape(NCORES, IMGS_PER_CORE, K * K, NBLK, 128, C_BLK)
        .transpose(0, 2, 4, 1, 3, 5))

    tf = list(FP16_W)
    Wf = np.zeros((G, len(tf), H, C_BLK), np.float16)
    for n, t in enumerate(tf):
        j = t % K
        Wf[:, n, :, j:j + W] = (Wc[:, t] * (1.0 / WSCALE)).astype(np.float16)
    wfprep = np.ascontiguousarray(
        Wf.reshape(NCORES, IMGS_PER_CORE, len(tf), NBLK, 128, C_BLK)
        .transpose(0, 2, 4, 1, 3, 5))

    ident = (np.eye(128) * WSCALE).astype(np.float16)
    in_maps = []
    for c in range(NCORES):
        in_maps.append({
            "ident": ident,
            "fin": fprep[c].reshape(K, 128, FREE),
            "win": wprep[c].reshape(K * K, 128, FREE),
            "win16": wfprep[c].reshape(len(tf), 128, FREE),
        })
    return in_maps


def kernel(frames, core, bias):
    global last_results
    from concourse.bass_utils import run_bass_kernel_spmd

    frames = np.asarray(frames, dtype=np.float32)
    core = np.asarray(core, dtype=np.float32)

    if "nc" not in _compiled:
        _compiled["nc"] = _build_nc()
    nc = _compiled["nc"]

    in_maps = _host_prep(frames, core)
    trace = os.environ.get("KC_TRACE") == "1"
    tmpdir = os.environ.get("KC_TRACE_DIR") or None
    if tmpdir:
        os.makedirs(tmpdir, exist_ok=True)
    res = run_bass_kernel_spmd(nc, in_maps, list(range(NCORES)), trace=trace,
                               tmpdir=tmpdir)
    last_results = res

    G = NCORES * IMGS_PER_CORE
    out = np.empty((G, H, W), np.float32)
    for c in range(NCORES):
        o = res.results[c]["oout"]  # [128, 4096] f16
        ov = o.reshape(128, IMGS_PER_CORE, NBLK, W).astype(np.float32)
        for img in range(IMGS_PER_CORE):
            out[c * IMGS_PER_CORE + img] = (
                ov[:, img].transpose(1, 0, 2).reshape(H, W))
    return out.reshape(4, 4, H, W)


# revision 14
# speedup vs baseline: 1.2350x; 1.0434x over previous
"""Per-pixel adaptive 5x5 conv (KPN) for Trainium2, 8-core data parallel.

out[g,h,w] = sum_{i,j} core[g,5i+j,h,w] * frames_pad[g,h+i-2,w+j-2]
with g = flattened (B,N) = 16 image planes; 2 planes per NeuronCore,
fused into one free dim so every elementwise op covers both.

v2 layout (vs v1's parity-copy scheme): each 128-row block stores 516
frame cols (out cols plus the +-2 halo), so ONE frame tile per row
shift i serves all 5 column taps j of its group: the column shift is
folded into the host weight layout (w'[c'] = w[c'-j]) and the PE reads
each product tile at free-dim offset j when accumulating bank b over
cols [b*516+j, b*516+j+512).  Frame DMA drops from 10.6 to 5.3 MB/core.

Engine split:
  DVE   - 25 products w_t*f_t (fp16 2x mode, ~2.2us each) plus the
          t=0 weight dequant (rides the ramp shadow)
  ACT   - 15 weight dequants (int8 codes -> fp16 copy, 3.6us each)
  DMA   - 9 weight tiles stored as fp16 codes in DRAM and loaded
          directly (no dequant anywhere; costs +0.53 MB of DMA each,
          cheaper than an engine cast while the queues have slack;
          SWDGE casting DMA was measured to cost read+write on the
          queues, worse than both)
  PE    - accumulates the 25 product streams into PSUM fp32 via
          matmuls against a stationary (2^-5 * I); 8 banks = [128,512]
  tail  - PSUM evac split scalar/vector engines, output stored in two
          0.5 MB chunks

Group 0 is split into per-image half tiles ([128,2064]) so the first
dequant+mul start as soon as ~0.25 MB lands instead of waiting for the
full 1.6 MB head (the DMA queues round-robin all in-flight transfers,
so first-tile latency is proportional to bytes in flight).

Weights are int8 (w8 = clip(round(w * 32), -127, 127)); the 2^-5
dequant scale is folded into the PE's stationary identity, so every
dequant path (ACT copy, DVE copy, casting DMA) is a pure cast.

Host layouts:
  fin [5, 128, 4128] fp16: fin[i][p, (img,blk,c)] =
     Fpad[img, blk*128+p+i, c], Fpad = pad(F, rows 2/2, cols 2/2),
     c in [0,516).
  win [25, 128, 4128] int8: win[t][p, (img,blk,c')] =
     clip(round(32*core[img, t, blk*128+p, c'-j]), -127, 127) for
     c'-j in [0,512) else 0, where j = t%5.
  oout [128, 4096] fp16 (host casts to f32).
"""

import os
import sys

import numpy as np

for _p in ("/opt/trn_rl_repo",):
    if _p not in sys.path and os.path.isdir(_p):
        sys.path.insert(0, _p)

K = 5
NCORES = 8
IMGS_PER_CORE = 2
H = W = 512
NBLK = 4          # 128-row blocks per image
C_BLK = 516       # 512 out cols + 4 halo cols (-2..513)
FREE = IMGS_PER_CORE * NBLK * C_BLK   # 4128
HFREE = FREE // 2                     # 2064 = one image
O_FREE = IMGS_PER_CORE * NBLK * W     # 4096
NBANK = 8
BANK = 512
WSCALE = 2.0 ** -5  # int8 weight dequant scale, folded into PE identity

# Dequant engine assignment per tap t = 5*i + j.
DVE_DEQ = frozenset()
FP16_W = (0, 1, 4, 8, 9, 13, 14, 18, 19, 23, 24)  # stored fp16, no dequant
FP16_IDX = {t: n for n, t in enumerate(FP16_W)}
# remaining 15 taps dequant on ACT

_compiled = {}
last_results = None  # BassKernelResults of the most recent run (for test.py)


def _build_nc():
    import concourse.bacc as bacc
    import concourse.mybir as mybir
    from concourse.tile import TileContext

    f16 = mybir.dt.float16
    f32 = mybir.dt.float32
    i8 = mybir.dt.int8

    nc = bacc.Bacc(None, target_bir_lowering=False, debug=False)
    ident = nc.dram_tensor("ident", [128, 128], f16, kind="ExternalInput")
    fin = nc.dram_tensor("fin", [K, 128, FREE], f16, kind="ExternalInput")
    win = nc.dram_tensor("win", [K * K, 128, FREE], i8, kind="ExternalInput")
    win16 = nc.dram_tensor("win16", [len(FP16_W), 128, FREE], f16,
                           kind="ExternalInput")
    oout = nc.dram_tensor("oout", [128, O_FREE], f16, kind="ExternalOutput")

    n_streams = K * K

    with TileContext(nc) as tc:
        with (
            tc.tile_pool(name="ipool", bufs=1) as ipool,
            tc.tile_pool(name="fpool", bufs=3) as fpool,
            tc.tile_pool(name="fhpool", bufs=1) as fhpool,
            tc.tile_pool(name="w8pool", bufs=2) as w8pool,
            tc.tile_pool(name="whpool", bufs=1) as whpool,
            tc.tile_pool(name="wpool", bufs=2) as wpool,
            tc.tile_pool(name="spool", bufs=3) as spool,
            tc.tile_pool(name="shpool", bufs=2) as shpool,
            tc.tile_pool(name="opool", bufs=1) as opool,
            tc.tile_pool(name="ppool", bufs=1, space="PSUM") as ppool,
        ):
            id_t = ipool.tile([128, 128], f16, tag="ident")

            banks = [ppool.tile([128, BANK], f32, tag=f"b{b}",
                                name=f"bank{b}")
                     for b in range(NBANK)]
            osb = opool.tile([128, O_FREE], f16, tag="osb")

            f_tiles = {}
            w8_tiles = {}
            w_tiles = {}
            bank_n = [0] * NBANK

            def pe_acc(tile, j, bank_list, off0):
                # rhs covers out cols of bank b at product offset j
                for lb, b in enumerate(bank_list):
                    s = bank_n[b]
                    bank_n[b] += 1
                    nc.tensor.matmul(
                        out=banks[b][:],
                        lhsT=id_t[:],
                        rhs=tile[:][:, off0 + lb * C_BLK + j:
                                    off0 + lb * C_BLK + j + BANK],
                        start=(s == 0),
                        stop=(s == n_streams - 1),
                    )

            def emit_w(tg, k):
                t = tg * K + k
                if t in FP16_IDX:
                    # fp16 codes straight from DRAM, no dequant step
                    w_t = wpool.tile([128, FREE], f16, tag=f"w{k}",
                                     name=f"wf16_{t}")
                    nc.sync.dma_start(out=w_t[:], in_=win16[FP16_IDX[t]])
                    w_tiles[t] = w_t
                    return
                w8_t = w8pool.tile([128, FREE], i8, tag=f"w8{k}",
                                   name=f"w8_{t}")
                nc.sync.dma_start(out=w8_t[:], in_=win[t])
                w8_tiles[t] = w8_t
                # ACT dequants are emitted here so ACT chases the DMA
                # arrivals a group ahead of the DVE muls; DVE's own
                # dequants are emitted inline in emit_compute.
                if t not in DVE_DEQ:
                    w_t = wpool.tile([128, FREE], f16, tag=f"w{k}",
                                     name=f"wdq{t}")
                    nc.scalar.copy(out=w_t[:], in_=w8_t[:])
                    w_tiles[t] = w_t

            def emit_compute(tg):
                for k in range(K):
                    t = tg * K + k
                    if t in DVE_DEQ:
                        w_t = wpool.tile([128, FREE], f16, tag=f"w{k}",
                                         name=f"wdq{t}")
                        nc.vector.tensor_copy(out=w_t[:],
                                              in_=w8_tiles[t][:])
                        w_tiles[t] = w_t
                    if t == n_streams - 1:
                        # final tap in halves so the PSUM evac + store
                        # tail starts ~1us earlier
                        for h in range(2):
                            sl = slice(h * HFREE, (h + 1) * HFREE)
                            tmp = shpool.tile([128, HFREE], f16, tag="sh")
                            nc.vector.tensor_mul(
                                out=tmp[:], in0=w_tiles[t][:][:, sl],
                                in1=f_tiles[tg][:][:, sl])
                            pe_acc(tmp, k, range(4 * h, 4 * h + 4), 0)
                        continue
                    tmp = spool.tile([128, FREE], f16, tag="s")
                    nc.vector.tensor_mul(out=tmp[:], in0=w_tiles[t][:],
                                         in1=f_tiles[tg][:])
                    pe_acc(tmp, k, range(NBANK), 0)

            def emit_f(tg):
                f_t = fpool.tile([128, FREE], f16, tag="f",
                                 name=f"fr{tg}")
                nc.sync.dma_start(out=f_t[:], in_=fin[tg])
                f_tiles[tg] = f_t

            # ---- head: group 0 in per-image halves for a fast ramp;
            # t0 is fp16-direct so the first mul is DMA-gated only ----
            wh16 = []
            fh = []
            for h in range(2):
                wh16.append(whpool.tile([128, HFREE], f16, tag=f"wh{h}",
                                        name=f"wh16_{h}"))
                fh.append(fhpool.tile([128, HFREE], f16, tag=f"fh{h}",
                                      name=f"fh{h}"))
            i0 = FP16_IDX[0]
            nc.sync.dma_start(out=wh16[0][:], in_=win16[i0][:, :HFREE])
            nc.sync.dma_start(out=fh[0][:], in_=fin[0][:, :HFREE])
            nc.sync.dma_start(out=wh16[1][:], in_=win16[i0][:, HFREE:])
            nc.sync.dma_start(out=fh[1][:], in_=fin[0][:, HFREE:])

            # DMAs globally ordered by consumption deadline: ACT-cast
            # (int8) taps get issued ~a group ahead of the fp16-direct
            # taps, since their arrival is followed by a 3.6us serial
            # cast on ACT before the DVE mul can consume them.
            emit_w(0, 2)
            emit_w(0, 3)
            emit_w(0, 1)
            emit_w(1, 0)
            emit_w(1, 1)
            emit_w(0, 4)
            emit_w(1, 2)
            emit_f(1)
            # identity rides the (empty) ACT hwdge ring, keeping its 128
            # tiny descriptors out of the sync ring's ramp window
            nc.scalar.dma_start(out=id_t[:], in_=ident[:])

            # group 0 compute: 10 half-muls, no casts anywhere
            for k in range(K):
                for h in range(2):
                    if k == 0:
                        w_ap = wh16[h][:]
                    else:
                        w_ap = w_tiles[k][:][:, h * HFREE:(h + 1) * HFREE]
                    tmp = shpool.tile([128, HFREE], f16, tag="sh")
                    nc.vector.tensor_mul(out=tmp[:], in0=w_ap,
                                         in1=fh[h][:])
                    pe_acc(tmp, k, range(4 * h, 4 * h + 4), 0)

            emit_w(2, 0)
            emit_w(1, 3)
            emit_w(2, 1)
            emit_w(1, 4)
            emit_w(2, 2)
            emit_f(2)
            emit_compute(1)
            emit_w(3, 0)
            emit_w(2, 3)
            emit_w(3, 1)
            emit_w(2, 4)
            emit_w(3, 2)
            emit_f(3)
            emit_compute(2)
            emit_w(4, 0)
            emit_w(3, 3)
            emit_w(4, 1)
            emit_w(3, 4)
            emit_w(4, 2)
            emit_f(4)
            emit_compute(3)
            emit_w(4, 3)
            emit_w(4, 4)
            emit_compute(K - 1)

            # ---- tail: PSUM fp32 -> SBUF fp16 split across scalar and
            # vector engines, store output in two chunks ----
            for b in range(NBANK):
                dst = osb[:][:, b * BANK:(b + 1) * BANK]
                if b % 2 == 0:
                    nc.scalar.copy(out=dst, in_=banks[b][:])
                else:
                    nc.vector.tensor_copy(out=dst, in_=banks[b][:])
                if b % 2 == 1:
                    c0 = (b - 1) * BANK
                    nc.sync.dma_start(out=oout[:, c0:c0 + 2 * BANK],
                                      in_=osb[:][:, c0:c0 + 2 * BANK])
    nc.finalize()
    return nc


def _host_prep(frames, core):
    """Build per-core in_maps. frames [4,4,1,512,512] f32, core [4,4,25,1,512,512]."""
    G = NCORES * IMGS_PER_CORE  # 16
    F = np.ascontiguousarray(frames.reshape(G, H, W))
    Wc = core.reshape(G, K * K, H, W)

    # frames: pad rows 2/2, cols 2/2 -> [G, 516, 516]
    Fp = np.pad(F, ((0, 0), (2, 2), (2, 2))).astype(np.float16)
    # A[g, i, blk, p, c] = Fp[g, blk*128+p+i, c]
    A = np.empty((G, K, NBLK, 128, C_BLK), np.float16)
    for i in range(K):
        A[:, i] = Fp[:, i:i + H, :].reshape(G, NBLK, 128, C_BLK)
    # fin[core][i, p, (img, blk, c)]
    fprep = np.ascontiguousarray(
        A.reshape(NCORES, IMGS_PER_CORE, K, NBLK, 128, C_BLK)
        .transpose(0, 2, 4, 1, 3, 5))

    # weights: codes of w/2^-5, column-shifted by j so products line up
    # with an aligned frame read; PE reads the product at offset j.
    # int8-rounded codes for the engine-cast taps, full fp16 codes for
    # the direct-load taps.
    w8 = np.clip(np.round(Wc * (1.0 / WSCALE)), -127, 127).astype(np.int8)
    Ws = np.zeros((G, K * K, H, C_BLK), np.int8)
    for j in range(K):
        Ws[:, j::K, :, j:j + W] = w8[:, j::K]
    wprep = np.ascontiguousarray(
        Ws.reshape(NCORES, IMGS_PER_CORE, K * K, NBLK, 128, C_BLK)
        .transpose(0, 2, 4, 1, 3, 5))

    tf = list(FP16_W)
    Wf = np.zeros((G, len(tf), H, C_BLK), np.float16)
    for n, t in enumerate(tf):
        j = t % K
        Wf[:, n, :, j:j + W] = (Wc[:, t] * (1.0 / WSCALE)).astype(np.float16)
    wfprep = np.ascontiguousarray(
        Wf.reshape(NCORES, IMGS_PER_CORE, len(tf), NBLK, 128, C_BLK)
        .transpose(0, 2, 4, 1, 3, 5))

    ident = (np.eye(128) * WSCALE).astype(np.float16)
    in_maps = []
    for c in range(NCORES):
        in_maps.append({
            "ident": ident,
            "fin": fprep[c].reshape(K, 128, FREE),
            "win": wprep[c].reshape(K * K, 128, FREE),
            "win16": wfprep[c].reshape(len(tf), 128, FREE),
        })
    return in_maps


def kernel(frames, core, bias):
    global last_results
    from concourse.bass_utils import run_bass_kernel_spmd

    frames = np.asarray(frames, dtype=np.float32)
    core = np.asarray(core, dtype=np.float32)

    if "nc" not in _compiled:
        _compiled["nc"] = _build_nc()
    nc = _compiled["nc"]

    in_maps = _host_prep(frames, core)
    trace = os.environ.get("KC_TRACE") == "1"
    tmpdir = os.environ.get("KC_TRACE_DIR") or None
    if tmpdir:
        os.makedirs(tmpdir, exist_ok=True)
    res = run_bass_kernel_spmd(nc, in_maps, list(range(NCORES)), trace=trace,
                               tmpdir=tmpdir)
    last_results = res

    G = NCORES * IMGS_PER_CORE
    out = np.empty((G, H, W), np.float32)
    for c in range(NCORES):
        o = res.results[c]["oout"]  # [128, 4096] f16
        ov = o.reshape(128, IMGS_PER_CORE, NBLK, W).astype(np.float32)
        for img in range(IMGS_PER_CORE):
            out[c * IMGS_PER_CORE + img] = (
                ov[:, img].transpose(1, 0, 2).reshape(H, W))
    return out.reshape(4, 4, H, W)
